# revision 1
# baseline (speedup 1.0000x reference)
"""CEAlignment Trainium2 kernel (8 NeuronCores, SPMD).

Sharding:
  - Phase 1 (MLPs): batch-data-parallel. Core c (c=0..7) runs MLP (c//4)
    [0 -> mlp1 on x1, 1 -> mlp2 on x2] on batch rows [(c%4)*128, +128).
    Activations stay batch-major [128, 2048] in SBUF; the stationary matmul
    operand is the transposed activation (PE transposes between layers; the
    initial x^T is prepared on host). Weights stream from HBM as the moving
    operand in float32r (FP22 multiply, fp32 accumulate, full PE rate at
    N=512). Biases are folded in as K=1 ones-row matmuls.
  - Phase 2: head_normalize (ddof=1) per 512-wide label block, transpose,
    then ONE AllToAll redistributes q^T so core c receives, at static
    addresses, label (c%4)'s full q1^T [512e x 512a] and q2^T [512e x 512b].
  - Phase 3: align = exp(q1_h @ q2_h^T / sqrt(E)) and a branchless Sinkhorn
    (2 unrolled iterations, convergence-flag blending identical to the
    reference's early-exit semantics; convergence checks use the squared
    form (d^2 <= ATOL^2) since |.| is not a DVE ALU op). Cross-partition
    sums use ones-column matmuls; partition broadcasts use gpsimd. Core c
    outputs the full [512, 512] matrix of label c%4 (cores 4-7 redundant).

W_MODE selects the weight-stream precision: "f32r" (fp32 weights, FP22
multiply — rel err ~5e-4, DMA-bound) or "bf16" (half the weight DMA,
hi/lo-split bf16 activations — rel err ~4.5e-3, ~25%% faster).
"""

import math
from contextlib import ExitStack

import numpy as np

import concourse.bacc as bacc
import concourse.bass as bass
import concourse.tile as tile
from concourse import mybir
from concourse.alu_op_type import AluOpType
from concourse.bass_utils import run_bass_kernel_spmd

# NOTE: TRN2 instructions may carry at most one sync wait; Bacc.compile()
# legalizes multi-wait instructions via generated event semaphores, so the
# program must be built on bacc.Bacc (not raw bass.Bass).

F32 = mybir.dt.float32
F32R = mybir.dt.float32r
AX = mybir.AxisListType.X
AF = mybir.ActivationFunctionType

B = 512          # batch (both sides)
D = 2048         # input dim
HD = 2048        # hidden dim
E = 512          # embed dim per label
L = 4            # num labels
R = 128          # batch rows per core
NCORES = 8
EPS = 1e-8
ATOL = 0.01
ISQ = 1.0 / math.sqrt(E)
SINKHORN_ITERS = 2

# moving-operand dtype for the MLP weight matmuls ("f32r" or "bf16")
W_MODE = "f32r"


def _r(ap):
    """Matmul-operand view (tiles are allocated as float32r already)."""
    return ap


def _emit(nc, tc, ctx, t):
    """Emit the SPMD program. `t` holds DRAM tensor handles."""
    w_dt = t["w0"].dtype  # weight dram dtype (f32 or bf16)

    def wview(ap):
        return ap

    NK = D // 128   # 16 k-tiles
    NN = HD // 512  # 4 n-tiles

    const_p = ctx.enter_context(tc.tile_pool(name="const", bufs=1))
    norm_p = ctx.enter_context(tc.tile_pool(name="norm", bufs=1))
    dram_p = ctx.enter_context(
        tc.tile_pool(name="dram", bufs=1, space=bass.MemorySpace.DRAM))
    ident = const_p.tile([128, 128], F32)
    nc.sync.dma_start(ident[:], t["ident"].ap())
    ones_sb = const_p.tile([128, 128], F32R)
    nc.sync.dma_start(ones_sb[:], t["ones"].ap())
    ones_row = ones_sb[0:1, :]
    ones_col = ones_sb[:, 0:1]
    p1m = const_p.tile([128, L], F32)   # p1 col for this core's label, tiled
    nc.sync.dma_start(p1m[:], t["p1m"].ap())
    p2r = const_p.tile([1, B], F32)     # p2 col for this core's label, row
    nc.sync.dma_start(p2r[:], t["p2r"].ap())
    epsb = const_p.tile([128, 1], F32)
    nc.vector.memset(epsb[:], EPS)

    qnT = norm_p.tile([128, L * E], F32R, tag="qnT")

    # ---------------- phase 1: MLP ----------------
    with ExitStack() as p1ctx:
        actT_p = p1ctx.enter_context(tc.tile_pool(name="actT", bufs=2))
        act_p = p1ctx.enter_context(tc.tile_pool(name="act", bufs=2))
        w_p = p1ctx.enter_context(tc.tile_pool(name="w", bufs=4))
        bias_p = p1ctx.enter_context(tc.tile_pool(name="bias", bufs=1))
        ps_mm = p1ctx.enter_context(
            tc.tile_pool(name="ps_mm", bufs=4, space=bass.MemorySpace.PSUM))
        ps_t = p1ctx.enter_context(
            tc.tile_pool(name="ps_t", bufs=2, space=bass.MemorySpace.PSUM))

        BF16 = mybir.dt.bfloat16
        if w_dt == BF16:
            # bf16 weights (half DMA) + hi/lo bf16 split of the stationary
            # activations (~16-bit effective mantissa, no DMA cost).
            actT_hi = actT_p.tile([128, D], BF16, tag="actT_hi")
            actT_lo = actT_p.tile([128, D], BF16, tag="actT_lo")
            nc.sync.dma_start(
                actT_hi[:].rearrange("p (j c) -> p j c", c=128),
                t["xT_hi"].ap().rearrange("(j p) c -> p j c", p=128))
            nc.sync.dma_start(
                actT_lo[:].rearrange("p (j c) -> p j c", c=128),
                t["xT_lo"].ap().rearrange("(j p) c -> p j c", p=128))
            actT_pair = (actT_hi, actT_lo)
        else:
            actT = actT_p.tile([128, D], F32R, tag="actT")
            nc.sync.dma_start(
                actT[:].rearrange("p (j c) -> p j c", c=128),
                t["xT"].ap().rearrange("(j p) c -> p j c", p=128))

        for lyr in range(4):
            act_out = act_p.tile([128, HD], F32, tag="act")
            w_dram = t[f"w{lyr}"].ap()
            bias_sb = bias_p.tile([1, HD], F32R, tag="bias")
            nc.sync.dma_start(bias_sb[:], t[f"b{lyr}"].ap())
            for n in range(NN):
                ps = ps_mm.tile([128, 512], F32, tag="mm")
                for kh in range(2):      # two half-k weight groups per n
                    wg = w_p.tile([128, 8 * 512], w_dt, tag="w")
                    # host-tiled layout: one contiguous 16KB run per partition
                    nc.sync.dma_start(wg[:], w_dram[n, kh])
                    for kk in range(8):
                        k = kh * 8 + kk
                        wgk = wg[:, kk * 512:(kk + 1) * 512]
                        if w_dt == BF16:
                            nc.tensor.matmul(
                                ps[:], actT_hi[:, k * 128:(k + 1) * 128],
                                wgk, start=(k == 0), stop=False)
                            nc.tensor.matmul(
                                ps[:], actT_lo[:, k * 128:(k + 1) * 128],
                                wgk, start=False, stop=False)
                        else:
                            nc.tensor.matmul(
                                ps[:], _r(actT[:, k * 128:(k + 1) * 128]),
                                wgk, start=(k == 0), stop=False)
                # bias via K=1 matmul: ones_row^T @ b_row
                nc.tensor.matmul(
                    ps[:], _r(ones_row),
                    _r(bias_sb[0:1, n * 512:(n + 1) * 512]),
                    start=False, stop=True)
                if lyr < 3:
                    nc.scalar.activation(act_out[:, n * 512:(n + 1) * 512],
                                         ps[:], AF.Relu)
                else:
                    nc.scalar.copy(act_out[:, n * 512:(n + 1) * 512], ps[:])
            if lyr < 3:
                if w_dt == BF16:
                    actT_hi = actT_p.tile([128, D], BF16, tag="actT_hi")
                    actT_lo = actT_p.tile([128, D], BF16, tag="actT_lo")
                    for j in range(NK):
                        pt = ps_t.tile([128, 128], F32, tag="t")
                        nc.tensor.transpose(
                            pt[:], act_out[:, j * 128:(j + 1) * 128],
                            ident[:])
                        hi = actT_hi[:, j * 128:(j + 1) * 128]
                        nc.vector.tensor_copy(hi, pt[:])
                        nc.vector.tensor_tensor(
                            actT_lo[:, j * 128:(j + 1) * 128], pt[:], hi,
                            AluOpType.subtract)
                else:
                    actT_next = actT_p.tile([128, D], F32R, tag="actT")
                    for j in range(NK):
                        pt = ps_t.tile([128, 128], F32, tag="t")
                        nc.tensor.transpose(
                            pt[:], act_out[:, j * 128:(j + 1) * 128],
                            ident[:])
                        nc.vector.tensor_copy(
                            actT_next[:, j * 128:(j + 1) * 128], pt[:])
                    actT = actT_next
            else:
                q = act_out  # [128, 2048] = [128 rows, L*E]

        # ---------- phase 2a: head_normalize (ddof=1) + transpose ----------
        qn = norm_p.tile([128, L * E], F32, tag="qn")
        for h in range(L):
            blk = q[:, h * E:(h + 1) * E]
            ssum = norm_p.tile([128, 1], F32, tag="s1")
            nc.vector.reduce_sum(ssum[:], blk, axis=AX)
            nmean = norm_p.tile([128, 1], F32, tag="s2")
            nc.vector.tensor_scalar(nmean[:], ssum[:], -1.0 / E, None,
                                    AluOpType.mult)
            scr = norm_p.tile([128, E], F32, tag="scr")
            ss = norm_p.tile([128, 1], F32, tag="s3")
            nc.scalar.activation(scr[:], blk, AF.Square, bias=nmean[:],
                                 scale=1.0, accum_out=ss[:])
            # std = sqrt(ss/(E-1) + eps); rstd = 1/std (exact reciprocal)
            std = norm_p.tile([128, 1], F32, tag="s4")
            nc.scalar.activation(std[:], ss[:], AF.Sqrt, bias=epsb[:],
                                 scale=1.0 / (E - 1))
            rstd = norm_p.tile([128, 1], F32, tag="s5")
            nc.vector.reciprocal(rstd[:], std[:])
            nc.vector.tensor_scalar(qn[:, h * E:(h + 1) * E], blk, nmean[:],
                                    rstd[:], AluOpType.add, AluOpType.mult)

        for j in range(NK):
            pt = ps_t.tile([128, 128], F32, tag="t")
            nc.tensor.transpose(pt[:], qn[:, j * 128:(j + 1) * 128], ident[:])
            nc.vector.tensor_copy(qnT[:, j * 128:(j + 1) * 128], pt[:])

    # ---------- phase 2b: AllToAll ----------
    # Send buffer [2 * L*E, 128]: two stacked copies of q^T; chunk j
    # (rows [512j, 512j+512)) goes to rank j and is label (j%4)'s e-block.
    cc_in = dram_p.tile([2 * L * E, R], F32R, tag="cc_in")
    for rep in range(2):
        nc.sync.dma_start(
            cc_in[rep * L * E:(rep + 1) * L * E, :]
            .rearrange("(j r) c -> r j c", r=128),
            qnT[:].rearrange("p (j c) -> p j c", c=128))
    cc_out = dram_p.tile([NCORES * E, R], F32R, tag="cc_out")
    if nc.num_devices == 1:
        # single-core cost-model build: stand in for the AllToAll
        nc.sync.dma_start(cc_out[0:L * E, :], cc_in[0:L * E, :])
        nc.sync.dma_start(cc_out[L * E:2 * L * E, :], cc_in[0:L * E, :])
    else:
        nc.gpsimd.collective_compute(
            "AllToAll", AluOpType.bypass,
            replica_groups=[list(range(NCORES))],
            ins=[cc_in[:].opt()], outs=[cc_out[:].opt()])

    # ---------------- phase 3: alignment + sinkhorn ----------------
    snk_p = ctx.enter_context(tc.tile_pool(name="snk", bufs=1))
    ps_a = ctx.enter_context(
        tc.tile_pool(name="ps_a", bufs=4, space=bass.MemorySpace.PSUM))
    ps_s = ctx.enter_context(
        tc.tile_pool(name="ps_s", bufs=1, space=bass.MemorySpace.PSUM))
    ps_sc = ctx.enter_context(
        tc.tile_pool(name="ps_sc", bufs=2, space=bass.MemorySpace.PSUM))

    # q1T: lhsT tiles; [:, (4a+e)*128] = [e-tile of label, a-chunk a]
    q1T = snk_p.tile([128, 2048], F32R, tag="q1T")
    nc.sync.dma_start(
        q1T[:].rearrange("p (a e c) -> p a e c", a=4, e=4),
        cc_out[0:L * E, :].rearrange("(a e p) c -> p a e c", e=4, p=128))
    # q2T: rhs tiles; [:, e*512 + rb*128] = [e-tile, b-chunk rb]
    q2T = snk_p.tile([128, 2048], F32R, tag="q2T")
    for rb in range(4):
        nc.sync.dma_start(
            q2T[:].rearrange("p (e rb c) -> p e rb c", e=4, rb=4)[:, :, rb],
            cc_out[(4 + rb) * E:(5 + rb) * E, :]
            .rearrange("(e p) c -> p e c", p=128))

    # align: A = exp((q1_h @ q2_h^T) / sqrt(E)); 4 a-tiles in one wide tile
    cur = snk_p.tile([128, 2048], F32R, tag="cur0")
    for a in range(4):
        ps = ps_a.tile([128, 512], F32, tag="al")
        for e in range(4):
            nc.tensor.matmul(
                ps[:], _r(q1T[:, (4 * a + e) * 128:(4 * a + e + 1) * 128]),
                _r(q2T[:, e * 512:(e + 1) * 512]),
                start=(e == 0), stop=(e == 3))
        nc.scalar.activation(cur[:, a * 512:(a + 1) * 512], ps[:], AF.Exp,
                             scale=ISQ)

    done_prev = None   # [1,1] flag: converged in an earlier iteration
    prev_out = None    # wide tile to keep if done_prev
    for it in range(SINKHORN_ITERS):
        # ---- column normalize: m1 = cur * (p2 / (colsum + eps)) ----
        pc = ps_s.tile([1, 512], F32, tag="cs")
        for a in range(4):
            nc.tensor.matmul(pc[:], _r(ones_col),
                             _r(cur[:, a * 512:(a + 1) * 512]),
                             start=(a == 0), stop=(a == 3))
        cse = snk_p.tile([1, 512], F32, tag="cse")
        nc.vector.tensor_scalar(cse[:], pc[:], EPS, None, AluOpType.add)
        csr = snk_p.tile([1, 512], F32, tag="csr")
        nc.vector.reciprocal(csr[:], cse[:])
        srow = snk_p.tile([1, 512], F32, tag="srow")
        nc.vector.tensor_tensor(srow[:], csr[:], p2r[:], AluOpType.mult)
        sful = snk_p.tile([128, 512], F32, tag="sful")
        nc.gpsimd.partition_broadcast(sful[:], srow[:])
        m1 = snk_p.tile([128, 2048], F32, tag="m1")
        rs4 = snk_p.tile([128, 4], F32, tag="rs4")
        for a in range(4):
            nc.vector.scalar_tensor_tensor(
                m1[:, a * 512:(a + 1) * 512], cur[:, a * 512:(a + 1) * 512],
                1.0, sful[:], AluOpType.mult, AluOpType.mult,
                accum_out=rs4[:, a:a + 1])
        # ---- row_ok: all (rowsum(m1) - p1)^2 <= ATOL^2 ----
        dev4 = snk_p.tile([128, 4], F32, tag="dev4")
        nc.vector.tensor_tensor(dev4[:], rs4[:], p1m[:], AluOpType.subtract)
        dev4sq = snk_p.tile([128, 4], F32, tag="dev4sq")
        nc.vector.tensor_tensor(dev4sq[:], dev4[:], dev4[:], AluOpType.mult)
        dev4r = snk_p.tile([128, 4], F32R, tag="dev4r")
        nc.vector.tensor_scalar(dev4r[:], dev4sq[:], ATOL * ATOL, 0.0,
                                AluOpType.subtract, AluOpType.max)
        pv = ps_sc.tile([1, 4], F32, tag="tiny")
        nc.tensor.matmul(pv[:], _r(ones_col), _r(dev4r[:]),
                         start=True, stop=True)
        vrow = snk_p.tile([1, 1], F32, tag="vrow")
        s14 = snk_p.tile([1, 4], F32, tag="s14")
        nc.vector.tensor_scalar(s14[:], pv[:], 0.0, None, AluOpType.add,
                                AluOpType.add, accum_out=vrow[:])
        grow = snk_p.tile([1, 1], F32, tag="grow")
        nc.vector.tensor_scalar(grow[:], vrow[:], 1e-30, None,
                                AluOpType.is_le)
        # ---- row normalize: m2 = m1 * (p1 / (rowsum + eps)) ----
        re4 = snk_p.tile([128, 4], F32, tag="re4")
        nc.vector.tensor_scalar(re4[:], rs4[:], EPS, None, AluOpType.add)
        rr4 = snk_p.tile([128, 4], F32, tag="rr4")
        nc.vector.reciprocal(rr4[:], re4[:])
        f4 = snk_p.tile([128, 4], F32, tag="f4")
        nc.vector.tensor_tensor(f4[:], rr4[:], p1m[:], AluOpType.mult)
        m2 = snk_p.tile([128, 2048], F32R, tag="m2")
        for a in range(4):
            nc.vector.tensor_scalar(m2[:, a * 512:(a + 1) * 512],
                                    m1[:, a * 512:(a + 1) * 512],
                                    f4[:, a:a + 1], None, AluOpType.mult)
        # ---- col_ok: all (colsum(m2) - p2)^2 <= ATOL^2 ----
        pc2 = ps_s.tile([1, 512], F32, tag="cs")
        for a in range(4):
            nc.tensor.matmul(pc2[:], _r(ones_col),
                             _r(m2[:, a * 512:(a + 1) * 512]),
                             start=(a == 0), stop=(a == 3))
        cd = snk_p.tile([1, 512], F32, tag="cd")
        nc.vector.tensor_tensor(cd[:], pc2[:], p2r[:], AluOpType.subtract)
        cd2 = snk_p.tile([1, 512], F32, tag="cd2")
        nc.vector.tensor_tensor(cd2[:], cd[:], cd[:], AluOpType.mult)
        cda = snk_p.tile([1, 512], F32, tag="cda")
        nc.vector.tensor_scalar(cda[:], cd2[:], ATOL * ATOL, None,
                                AluOpType.subtract)
        vcol = snk_p.tile([1, 1], F32, tag="vcol")
        cdr = snk_p.tile([1, 512], F32, tag="cdr")
        nc.vector.tensor_scalar(cdr[:], cda[:], 0.0, None, AluOpType.max,
                                AluOpType.add, accum_out=vcol[:])
        gcol = snk_p.tile([1, 1], F32, tag="gcol")
        nc.vector.tensor_scalar(gcol[:], vcol[:], 1e-30, None,
                                AluOpType.is_le)
        # ---- new = grow ? m1 : m2 (branchless, full-width ops) ----
        pg = snk_p.tile([128, 1], F32, tag="pg")
        nc.gpsimd.partition_broadcast(pg[:], grow[:])
        d = snk_p.tile([128, 2048], F32, tag="d")
        nc.vector.tensor_tensor(d[:], m1[:], m2[:], AluOpType.subtract)
        nw = snk_p.tile([128, 2048], F32R, tag=f"nw{it}")
        nc.vector.scalar_tensor_tensor(nw[:], d[:], pg[:], m2[:],
                                       AluOpType.mult, AluOpType.add)
        if it == 0:
            done_prev = snk_p.tile([1, 1], F32, tag="done")
            nc.vector.tensor_tensor(done_prev[:], grow[:], gcol[:],
                                    AluOpType.max)
            prev_out = nw
            cur = nw
        else:
            # final = done_prev ? prev_out : new
            pd = snk_p.tile([128, 1], F32, tag="pd")
            nc.gpsimd.partition_broadcast(pd[:], done_prev[:])
            d2 = snk_p.tile([128, 2048], F32, tag="d2")
            nc.vector.tensor_tensor(d2[:], prev_out[:], nw[:],
                                    AluOpType.subtract)
            fin = snk_p.tile([128, 2048], F32, tag="fin")
            nc.vector.scalar_tensor_tensor(fin[:], d2[:], pd[:], nw[:],
                                           AluOpType.mult, AluOpType.add)
            cur = fin

    # out[a*128 + r, c] = cur[r, a*512 + c] -- one DMA
    nc.sync.dma_start(
        t["out"].ap().rearrange("(a r) c -> r a c", r=128),
        cur[:].rearrange("p (a c) -> p a c", c=512))


def build_program(w_mode=W_MODE, num_devices=NCORES):
    w_dt = F32R if w_mode == "f32r" else mybir.dt.bfloat16
    nc = bacc.Bacc("TRN2", target_bir_lowering=False, debug=False,
                   num_devices=num_devices)
    t = {}
    if w_mode == "bf16":
        t["xT_hi"] = nc.dram_tensor("xT_hi", [D, R], mybir.dt.bfloat16,
                                    kind="ExternalInput")
        t["xT_lo"] = nc.dram_tensor("xT_lo", [D, R], mybir.dt.bfloat16,
                                    kind="ExternalInput")
    else:
        t["xT"] = nc.dram_tensor("xT", [D, R], F32R, kind="ExternalInput")
    for lyr in range(4):
        t[f"w{lyr}"] = nc.dram_tensor(f"w{lyr}", [HD // 512, 2, 128, 8 * 512],
                                      w_dt, kind="ExternalInput")
        t[f"b{lyr}"] = nc.dram_tensor(f"b{lyr}", [1, HD], F32R,
                                      kind="ExternalInput")
    t["p1m"] = nc.dram_tensor("p1m", [128, L], F32, kind="ExternalInput")
    t["p2r"] = nc.dram_tensor("p2r", [1, B], F32, kind="ExternalInput")
    t["ident"] = nc.dram_tensor("ident", [128, 128], F32,
                                kind="ExternalInput")
    t["ones"] = nc.dram_tensor("ones", [128, 128], F32R,
                               kind="ExternalInput")
    t["out"] = nc.dram_tensor("out", [B, B], F32, kind="ExternalOutput")

    with ExitStack() as ctx:
        tc = ctx.enter_context(tile.TileContext(nc))
        _emit(nc, tc, ctx, t)
    nc.compile()
    return nc


def make_in_maps(x1, x2, x1_probs, x2_probs, mlp1_ws, mlp1_bs, mlp2_ws,
                 mlp2_bs, w_mode=W_MODE):
    if w_mode == "f32r":
        w_np = np.float32
    else:
        import ml_dtypes
        w_np = ml_dtypes.bfloat16
    xT = [np.ascontiguousarray(np.asarray(x1, np.float32).T),
          np.ascontiguousarray(np.asarray(x2, np.float32).T)]
    def _tile_w(w):
        w = np.asarray(w, np.float32).astype(w_np)
        w = w.reshape(2, 8, 128, HD // 512, 512)     # [kh, kk, p, n, c]
        w = w.transpose(3, 0, 2, 1, 4)               # [n, kh, p, kk, c]
        return np.ascontiguousarray(w.reshape(HD // 512, 2, 128, 8 * 512))

    ws = [[_tile_w(w) for w in mlp1_ws], [_tile_w(w) for w in mlp2_ws]]
    bs = [[np.asarray(b, np.float32).reshape(1, HD) for b in mlp1_bs],
          [np.asarray(b, np.float32).reshape(1, HD) for b in mlp2_bs]]
    p1 = np.asarray(x1_probs, np.float32)
    p2 = np.asarray(x2_probs, np.float32)
    ident = np.eye(128, dtype=np.float32)
    in_maps = []
    for c in range(NCORES):
        m = c // 4          # which MLP
        rslice = c % 4      # which batch rows
        h = c % 4           # which label for sinkhorn
        xTc = np.ascontiguousarray(xT[m][:, rslice * R:(rslice + 1) * R])
        if w_mode == "bf16":
            import ml_dtypes
            hi = xTc.astype(ml_dtypes.bfloat16)
            lo = (xTc - hi.astype(np.float32)).astype(ml_dtypes.bfloat16)
            xin = {"xT_hi": hi, "xT_lo": lo}
        else:
            xin = {"xT": xTc}
        d = {**xin,
             "p1m": np.ascontiguousarray(p1[:, h].reshape(4, 128).T),
             "p2r": np.ascontiguousarray(p2[:, h].reshape(1, B)),
             "ident": ident,
             "ones": np.ones((128, 128), np.float32)}
        for lyr in range(4):
            d[f"w{lyr}"] = ws[m][lyr]
            d[f"b{lyr}"] = bs[m][lyr]
        in_maps.append(d)
    return in_maps


_PROGRAM_CACHE = {}


def kernel(x1, x2, x1_probs, x2_probs, mlp1_ws, mlp1_bs, mlp2_ws, mlp2_bs,
           **run_kwargs):
    if W_MODE not in _PROGRAM_CACHE:
        _PROGRAM_CACHE[W_MODE] = build_program(W_MODE)
    nc = _PROGRAM_CACHE[W_MODE]
    in_maps = make_in_maps(x1, x2, x1_probs, x2_probs, mlp1_ws, mlp1_bs,
                           mlp2_ws, mlp2_bs)
    res = run_bass_kernel_spmd(nc, in_maps, core_ids=list(range(NCORES)),
                               **run_kwargs)
    out = np.stack([res.results[h]["out"] for h in range(L)], axis=2)
    kernel.last_results = res
    return np.ascontiguousarray(out.astype(np.float32))



# revision 7
# speedup vs baseline: 1.4354x; 1.4354x over previous
"""CEAlignment Trainium2 kernel (8 NeuronCores, SPMD).

Sharding (v2, N-sharded MLPs with weight-stationary dataflow):
  - Phase 1 (MLPs): each MLP's weights are column-sharded across its 4 cores
    (core c: MLP c//4, output columns [512*(c%4), +512) of every layer), so
    no core duplicates weight traffic (8.4 MB bf16/core vs 67 MB f32 for the
    data-parallel layout). Weights are the stationary matmul operand
    ([k,n] tiles); activations stay in [feature-part, batch-free] layout the
    whole way through, so there are NO inter-layer transposes. Biases are
    folded in as K=1 matmuls (ones-row moving operand). Activations move
    between layers via a half-batch-pipelined AllGather (groups {0-3},{4-7})
    in bf16, overlapped with compute on the other half.
  - Phase 2: the 512-wide column shard of layer 3 is exactly one label's
    embedding block, so core c already holds label (c%4)'s full q for its
    side. head_normalize is folded into the alignment gram:
    (q1-m1)·(q2-m2) = G - S1*S2/E, scaled by r1*r2 post-matmul. Cores c and
    c+4 exchange raw q (bf16) + (neg-mean, rstd) stat rows via pair
    AllGathers (groups {c, c+4}).
  - Phase 3: align = exp(fixup(G)/sqrt(E)) and a branchless 2-iteration
    Sinkhorn that reproduces the reference's early-exit semantics with the
    convergence selects folded into the per-row/per-col normalization
    factors (g = done ? 1 : factor), so each iteration is only two
    full-width bf16 DVE passes. Cross-partition sums and broadcasts use
    ones-matmuls on the PE. Pair cores compute the same label redundantly;
    cores 0-3's outputs are gathered on the host.

The num_devices==1 build (used by the cost-model timeline) replaces each
collective with local DMAs of equivalent size, as in the v1 kernel.
"""

import math
from contextlib import ExitStack

import numpy as np

import concourse.bacc as bacc
import concourse.bass as bass
import concourse.tile as tile
from concourse import mybir
from concourse.alu_op_type import AluOpType
from concourse.bass_utils import run_bass_kernel_spmd

F32 = mybir.dt.float32
F32R = mybir.dt.float32r
BF16 = mybir.dt.bfloat16
AF = mybir.ActivationFunctionType

B = 512          # batch (both sides)
D = 2048         # input dim
HD = 2048        # hidden dim
E = 512          # embed dim per label
L = 4            # num labels
NCORES = 8
NK = 16          # contraction chunks of 128
NS = 4           # n-tiles of 128 in this core's 512-wide column shard
HB = 256         # half-batch pipeline granule
EPS = 1e-8
ATOL = 0.01
ISQ = 1.0 / math.sqrt(E)
SINKHORN_ITERS = 2
W_MODE = "bf16"  # kept for the test harness printout

LAYER_GROUPS = [[0, 1, 2, 3], [4, 5, 6, 7]]
PAIR_GROUPS = [[0, 4], [1, 5], [2, 6], [3, 7]]


def _allgather(nc, in_ap, out_ap, groups, nrep):
    """AllGather, or equivalent-size local DMAs on the 1-device build."""
    if nc.num_devices == 1:
        n = out_ap.shape[0] // nrep
        for r in range(nrep):
            nc.sync.dma_start(out_ap[r * n:(r + 1) * n], in_ap)
    else:
        nc.gpsimd.collective_compute(
            "AllGather", AluOpType.bypass, replica_groups=groups,
            ins=[in_ap.opt()], outs=[out_ap.opt()])


def _emit(nc, tc, ctx, t):
    const_p = ctx.enter_context(tc.tile_pool(name="const", bufs=1))
    dram_p = ctx.enter_context(
        tc.tile_pool(name="dram", bufs=1, space=bass.MemorySpace.DRAM))

    ones_sb = const_p.tile([128, HB], F32R)
    nc.sync.dma_start(ones_sb[:], t["ones"].ap())
    brow = const_p.tile([1, 4 * 512], F32R)       # per-layer bias rows
    nc.sync.dma_start(brow[:], t["brow"].ap())
    cst = const_p.tile([1, 2], F32R)              # [-512, 1]
    nc.sync.dma_start(cst[:], t["cst"].ap())
    p1m = const_p.tile([128, L], F32)             # p1 col, chunk-major
    nc.sync.dma_start(p1m[:], t["p1m"].ap())
    p2r = const_p.tile([1, B], F32)               # p2 col as a row
    nc.sync.dma_start(p2r[:], t["p2r"].ap())
    epsb = const_p.tile([1, 1], F32)
    nc.vector.memset(epsb[:], EPS)

    ones_col = ones_sb[:, 0:1]                    # [128,1] lhsT: partition sum
    ones_k1 = ones_sb[0:1, 0:128]                 # [1,128] lhsT: bcast to parts

    # DRAM exchange buffers
    ag_in = [[dram_p.tile([E, HB], BF16, tag=f"agi{l}_{h}", name=f"agi{l}_{h}")
              for h in range(2)] for l in range(3)]
    ag_out = [[dram_p.tile([HD, HB], BF16, tag=f"ago{l}_{h}", name=f"ago{l}_{h}")
               for h in range(2)] for l in range(3)]
    pq_in = [dram_p.tile([E, HB], BF16, tag=f"pqi{h}", name=f"pqi{h}")
             for h in range(2)]
    pq_out = [dram_p.tile([2 * E, HB], BF16, tag=f"pqo{h}", name=f"pqo{h}")
              for h in range(2)]
    st_in = dram_p.tile([1, 1024], F32R, tag="sti")
    st_out = dram_p.tile([2, 1024], F32R, tag="sto")

    q1_sb = const_p.tile([128, 4 * B], BF16, tag="q1")   # [e-chunk, batch]
    q2_sb = const_p.tile([128, 4 * B], BF16, tag="q2")
    qh = [const_p.tile([128, NS * HB], BF16, tag=f"qh{h}", name=f"qh{h}")
          for h in range(2)]

    # ---------------- phase 1: MLPs ----------------
    with ExitStack() as p1:
        w_p = p1.enter_context(tc.tile_pool(name="w", bufs=2))
        act_p = p1.enter_context(tc.tile_pool(name="act", bufs=2))
        ps_mm = p1.enter_context(
            tc.tile_pool(name="ps_mm", bufs=6, space=bass.MemorySpace.PSUM))
        ps_q = p1.enter_context(
            tc.tile_pool(name="ps_q", bufs=1, space=bass.MemorySpace.PSUM))

        # x input, per half (rhs of layer 0)
        x_h = []
        for h in range(2):
            xt = act_p.tile([128, NK * HB], BF16, tag=f"x{h}")
            nc.sync.dma_start(
                xt[:].rearrange("p (k b) -> p k b", b=HB),
                t["x"].ap().rearrange("(k p) b -> p k b", p=128)
                [:, :, h * HB:(h + 1) * HB])
            x_h.append(xt)

        def load_w(lyr):
            w = w_p.tile([128, NK * NS * 128], BF16, tag="w")
            nc.sync.dma_start(w[:], t[f"w{lyr}"].ap())
            return w

        w_cur = load_w(0)
        rhs = x_h  # per-half rhs tiles, [128, NK*HB], chunk k at [k*HB,(k+1)*HB)

        for lyr in range(4):
            w_nxt = load_w(lyr + 1) if lyr < 3 else None
            rhs_nxt = []
            for h in range(2):
                if lyr < 3:
                    oo = act_p.tile([128, NS * HB], BF16, tag=f"oo{h}")
                else:
                    oo = qh[h]
                for n in range(NS):
                    ps = ps_mm.tile([128, HB], F32, tag="mm")
                    for k in range(NK):
                        nc.tensor.matmul(
                            ps[:], w_cur[:, (k * NS + n) * 128:(k * NS + n + 1) * 128],
                            rhs[h][:, k * HB:(k + 1) * HB],
                            start=(k == 0), stop=False)
                    nc.tensor.matmul(
                        ps[:], brow[0:1, lyr * 512 + n * 128:lyr * 512 + (n + 1) * 128],
                        ones_sb[0:1, 0:HB], start=False, stop=True)
                    nc.scalar.activation(
                        oo[:, n * HB:(n + 1) * HB], ps[:],
                        AF.Relu if lyr < 3 else AF.Copy)
                if lyr < 3:
                    # exchange this half's slice; peers' slices land in fa
                    nc.sync.dma_start(
                        ag_in[lyr][h][:].rearrange("(n p) b -> p n b", p=128),
                        oo[:].rearrange("p (n b) -> p n b", b=HB))
                    _allgather(nc, ag_in[lyr][h][:], ag_out[lyr][h][:],
                               LAYER_GROUPS, 4)
                    fa = act_p.tile([128, NK * HB], BF16, tag=f"fa{h}")
                    nc.sync.dma_start(
                        fa[:].rearrange("p (k b) -> p k b", b=HB),
                        ag_out[lyr][h][:].rearrange("(k p) b -> p k b", p=128))
                    rhs_nxt.append(fa)
                else:
                    # last layer: q slice out for the pair exchange
                    nc.sync.dma_start(
                        pq_in[h][:].rearrange("(n p) b -> p n b", p=128),
                        oo[:].rearrange("p (n b) -> p n b", b=HB))
                    _allgather(nc, pq_in[h][:], pq_out[h][:],
                               PAIR_GROUPS, 2)
            rhs = rhs_nxt
            w_cur = w_nxt

        # local head-norm stats: S = sum_e q, Q = sum_e q^2 (per batch col)
        s_ps = ps_q.tile([1, B], F32, tag="s")
        q_ps = ps_q.tile([1, B], F32, tag="q")
        for h in range(2):
            qsq = act_p.tile([128, NS * HB], BF16, tag=f"qsq{h}")
            nc.vector.tensor_tensor(qsq[:], qh[h][:], qh[h][:], AluOpType.mult)
            for e4 in range(NS):
                nc.tensor.matmul(s_ps[0:1, h * HB:(h + 1) * HB], ones_col,
                                 qh[h][:, e4 * HB:(e4 + 1) * HB],
                                 start=(e4 == 0), stop=(e4 == NS - 1))
                nc.tensor.matmul(q_ps[0:1, h * HB:(h + 1) * HB], ones_col,
                                 qsq[:, e4 * HB:(e4 + 1) * HB],
                                 start=(e4 == 0), stop=(e4 == NS - 1))
        # negm = -S/512 ; r = 1/sqrt((Q - S^2/512)/511 + eps)
        stat2 = const_p.tile([1, 1024], F32R, tag="stat2")
        negm = stat2[:, 0:512]
        rrow = stat2[:, 512:1024]
        nc.scalar.activation(negm, s_ps[:], AF.Copy, scale=-1.0 / E)
        s2row = const_p.tile([1, B], F32R, tag="s2row")
        nc.vector.tensor_tensor(s2row[:], s_ps[:], s_ps[:], AluOpType.mult)
        varr = const_p.tile([1, B], F32R, tag="varr")
        nc.vector.scalar_tensor_tensor(varr[:], s2row[:], -1.0 / E, q_ps[:],
                                       AluOpType.mult, AluOpType.add)
        sdr = const_p.tile([1, B], F32R, tag="sdr")
        nc.scalar.activation(sdr[:], varr[:], AF.Sqrt, bias=epsb[:],
                             scale=1.0 / (E - 1))
        with nc.allow_low_precision("rstd row feeds f32r matmul operands"):
            nc.vector.reciprocal(rrow, sdr[:])
        nc.sync.dma_start(st_in[:], stat2[:])
        _allgather(nc, st_in[:], st_out[:], PAIR_GROUPS, 2)

        # load gathered q into absolute [side] layout
        for h in range(2):
            for (dst, lo) in ((q1_sb, 0), (q2_sb, E)):
                nc.sync.dma_start(
                    dst[:].rearrange("p (e b) -> p e b", b=B)
                    [:, :, h * HB:(h + 1) * HB],
                    pq_out[h][lo:lo + E]
                    .rearrange("(e p) b -> p e b", p=128))

    # ---------------- phase 3: align + sinkhorn ----------------
    stc = const_p.tile([1, 2048], F32R, tag="stc")  # [negm1, r1, negm2, r2]
    nc.sync.dma_start(
        stc[:].rearrange("p (g c) -> p g c", c=1024),
        st_out[:].rearrange("(g p) c -> p g c", p=1))

    snk_p = ctx.enter_context(tc.tile_pool(name="snk", bufs=1))
    ps_g = ctx.enter_context(
        tc.tile_pool(name="ps_g", bufs=2, space=bass.MemorySpace.PSUM))
    ps_bc = ctx.enter_context(
        tc.tile_pool(name="ps_bc", bufs=3, space=bass.MemorySpace.PSUM))
    ps_sm = ctx.enter_context(
        tc.tile_pool(name="ps_sm", bufs=1, space=bass.MemorySpace.PSUM))
    tmp_p = ctx.enter_context(tc.tile_pool(name="tmp", bufs=2))

    # broadcasts of partner-side stats and column extracts of own-side stats
    negm2b = ps_bc.tile([128, B], F32, tag="bc")
    nc.tensor.matmul(negm2b[:], ones_k1, stc[0:1, 1024:1536],
                     start=True, stop=True)
    r2b = ps_bc.tile([128, B], F32, tag="bc")
    nc.tensor.matmul(r2b[:], ones_k1, stc[0:1, 1536:2048],
                     start=True, stop=True)
    misc = ps_sm.tile([128, 16], F32, tag="misc")
    colx = misc[:, 0:8]                           # S1 (a-chunk), r1 (a-chunk)
    for a in range(4):
        nc.tensor.matmul(colx[:, a:a + 1], stc[0:1, a * 128:(a + 1) * 128],
                         cst[0:1, 0:1], start=True, stop=True)
        nc.tensor.matmul(colx[:, 4 + a:5 + a],
                         stc[0:1, 512 + a * 128:512 + (a + 1) * 128],
                         cst[0:1, 1:2], start=True, stop=True)

    # align = exp(((G - S1*S2/E) * r1 * r2) / sqrt(E)), chunk a = batch1 tile
    cur = snk_p.tile([128, 4 * B], BF16, tag="cur")
    for a in range(4):
        g_ps = ps_g.tile([128, B], F32, tag="g")
        for e4 in range(4):
            nc.tensor.matmul(
                g_ps[:], q1_sb[:, e4 * B + a * 128:e4 * B + (a + 1) * 128],
                q2_sb[:, e4 * B:(e4 + 1) * B],
                start=(e4 == 0), stop=(e4 == 3))
        u = tmp_p.tile([128, B], F32R, tag="u")
        nc.vector.scalar_tensor_tensor(u[:], negm2b[:], colx[:, a:a + 1],
                                       g_ps[:], AluOpType.mult, AluOpType.add)
        v = tmp_p.tile([128, B], F32R, tag="v")
        nc.vector.scalar_tensor_tensor(v[:], u[:], colx[:, 4 + a:5 + a],
                                       r2b[:], AluOpType.mult, AluOpType.mult)
        nc.scalar.activation(cur[:, a * B:(a + 1) * B], v[:], AF.Exp,
                             scale=ISQ)

    # ---- sinkhorn: 2 iterations, reference-faithful early-exit blending ----
    def colsum(mat):
        ps = ps_sm.tile([1, B], F32, tag="cs")
        for a in range(4):
            nc.tensor.matmul(ps[:], ones_col, mat[:, a * B:(a + 1) * B],
                             start=(a == 0), stop=(a == 3))
        return ps

    def bcast_row(row_sb):
        ps = ps_bc.tile([128, B], F32, tag="bc")
        nc.tensor.matmul(ps[:], ones_k1, row_sb, start=True, stop=True)
        return ps

    _pcol = [12]

    def bcast_scalar(s11):
        ps = misc[:, _pcol[0]:_pcol[0] + 1]
        _pcol[0] += 1
        nc.tensor.matmul(ps, ones_k1, s11, start=True, stop=True)
        return ps

    def row_norm_factors(rs4, tag):
        """f4 = p1 / (rowsum + eps); grow = all-rows-converged flag [1,1]."""
        re4 = snk_p.tile([128, L], F32, tag=f"re{tag}")
        nc.vector.tensor_scalar(re4[:], rs4[:], EPS, None, AluOpType.add)
        rr4 = snk_p.tile([128, L], F32, tag=f"rr{tag}")
        nc.vector.reciprocal(rr4[:], re4[:])
        f4 = snk_p.tile([128, L], F32, tag=f"f4{tag}")
        nc.vector.tensor_tensor(f4[:], rr4[:], p1m[:], AluOpType.mult)
        dev = snk_p.tile([128, L], F32, tag=f"dv{tag}")
        nc.vector.tensor_tensor(dev[:], rs4[:], p1m[:], AluOpType.subtract)
        dsq = snk_p.tile([128, L], F32, tag=f"dq{tag}")
        nc.vector.tensor_tensor(dsq[:], dev[:], dev[:], AluOpType.mult)
        dcl = snk_p.tile([128, L], F32R, tag=f"dc{tag}")
        nc.vector.tensor_scalar(dcl[:], dsq[:], ATOL * ATOL, 0.0,
                                AluOpType.subtract, AluOpType.max)
        pv = misc[0:1, 8:12]
        nc.tensor.matmul(pv, ones_col, dcl[:], start=True, stop=True)
        vr = snk_p.tile([1, 1], F32, tag=f"vr{tag}")
        s14 = snk_p.tile([1, L], F32, tag=f"s14{tag}")
        nc.vector.tensor_scalar(s14[:], pv, 0.0, None, AluOpType.add,
                                AluOpType.add, accum_out=vr[:])
        grow = snk_p.tile([1, 1], F32R, tag=f"gr{tag}")
        nc.vector.tensor_scalar(grow[:], vr[:], 1e-30, None, AluOpType.is_le)
        return f4, grow

    # iteration 1: column normalize
    cs1 = colsum(cur)
    cse = snk_p.tile([1, B], F32, tag="cse")
    nc.vector.tensor_scalar(cse[:], cs1[:], EPS, None, AluOpType.add)
    csr = snk_p.tile([1, B], F32, tag="csr")
    nc.vector.reciprocal(csr[:], cse[:])
    srow = snk_p.tile([1, B], F32R, tag="srow")
    nc.vector.tensor_tensor(srow[:], csr[:], p2r[:], AluOpType.mult)
    sb_ps = bcast_row(srow[:])
    sful = snk_p.tile([128, B], BF16, tag="sful")
    nc.scalar.copy(sful[:], sb_ps[:])
    m1 = snk_p.tile([128, 4 * B], BF16, tag="m1")
    rs4 = snk_p.tile([128, L], F32, tag="rs4")
    for a in range(4):
        nc.vector.scalar_tensor_tensor(
            m1[:, a * B:(a + 1) * B], cur[:, a * B:(a + 1) * B], 1.0, sful[:],
            AluOpType.mult, AluOpType.mult, accum_out=rs4[:, a:a + 1])
    # row normalize, folded with the row_ok select: g1 = row_ok ? 1 : f4
    f4, grow = row_norm_factors(rs4, "1")
    pg = bcast_scalar(grow[:])
    d4 = snk_p.tile([128, L], F32, tag="d4")
    nc.vector.tensor_scalar(d4[:], f4[:], -1.0, 1.0, AluOpType.mult,
                            AluOpType.add)
    g1 = snk_p.tile([128, L], F32, tag="g1")
    nc.vector.scalar_tensor_tensor(g1[:], d4[:], pg, f4[:],
                                   AluOpType.mult, AluOpType.add)
    cur2 = snk_p.tile([128, 4 * B], BF16, tag="cur2")
    for a in range(4):
        nc.vector.tensor_scalar(cur2[:, a * B:(a + 1) * B],
                                m1[:, a * B:(a + 1) * B], g1[:, a:a + 1],
                                None, AluOpType.mult)

    # iteration 2 (no-op via factor blending if iteration 1 converged)
    cs2 = colsum(cur2)
    cd = snk_p.tile([1, B], F32, tag="cd")
    nc.vector.tensor_tensor(cd[:], cs2[:], p2r[:], AluOpType.subtract)
    cdq = snk_p.tile([1, B], F32, tag="cdq")
    nc.vector.tensor_tensor(cdq[:], cd[:], cd[:], AluOpType.mult)
    vc = snk_p.tile([1, 1], F32, tag="vc")
    cdc = snk_p.tile([1, B], F32, tag="cdc")
    nc.vector.tensor_scalar(cdc[:], cdq[:], ATOL * ATOL, 0.0,
                            AluOpType.subtract, AluOpType.max,
                            accum_out=vc[:])
    gcol = snk_p.tile([1, 1], F32, tag="gcol")
    nc.vector.tensor_scalar(gcol[:], vc[:], 1e-30, None, AluOpType.is_le)
    done1 = snk_p.tile([1, 1], F32R, tag="done1")
    nc.vector.tensor_tensor(done1[:], grow[:], gcol[:], AluOpType.max)
    pd = bcast_scalar(done1[:])
    cse2 = snk_p.tile([1, B], F32, tag="cse2")
    nc.vector.tensor_scalar(cse2[:], cs2[:], EPS, None, AluOpType.add)
    csr2 = snk_p.tile([1, B], F32, tag="csr2")
    nc.vector.reciprocal(csr2[:], cse2[:])
    srow2 = snk_p.tile([1, B], F32R, tag="srow2")
    nc.vector.tensor_tensor(srow2[:], csr2[:], p2r[:], AluOpType.mult)
    sb2_ps = bcast_row(srow2[:])
    ssb = snk_p.tile([128, B], BF16, tag="ssb")
    nc.scalar.copy(ssb[:], sb2_ps[:])
    # s' = done1 ? 1 : srow2  (full-width blend of the column factor)
    t1 = snk_p.tile([128, B], BF16, tag="t1")
    nc.vector.tensor_scalar(t1[:], ssb[:], -1.0, 1.0, AluOpType.mult,
                            AluOpType.add)
    sp = snk_p.tile([128, B], BF16, tag="sp")
    nc.vector.scalar_tensor_tensor(sp[:], t1[:], pd, ssb[:],
                                   AluOpType.mult, AluOpType.add)
    m1b = snk_p.tile([128, 4 * B], BF16, tag="m1b")
    rs4b = snk_p.tile([128, L], F32, tag="rs4b")
    for a in range(4):
        nc.vector.scalar_tensor_tensor(
            m1b[:, a * B:(a + 1) * B], cur2[:, a * B:(a + 1) * B], 1.0, sp[:],
            AluOpType.mult, AluOpType.mult, accum_out=rs4b[:, a:a + 1])
    f4b, grow2 = row_norm_factors(rs4b, "2")
    og = snk_p.tile([1, 1], F32R, tag="og")
    nc.vector.tensor_tensor(og[:], done1[:], grow2[:], AluOpType.max)
    pg2 = bcast_scalar(og[:])
    d4b = snk_p.tile([128, L], F32, tag="d4b")
    nc.vector.tensor_scalar(d4b[:], f4b[:], -1.0, 1.0, AluOpType.mult,
                            AluOpType.add)
    g2 = snk_p.tile([128, L], F32, tag="g2")
    nc.vector.scalar_tensor_tensor(g2[:], d4b[:], pg2, f4b[:],
                                   AluOpType.mult, AluOpType.add)
    fin = snk_p.tile([128, 4 * B], BF16, tag="fin")
    for a in range(4):
        nc.vector.tensor_scalar(fin[:, a * B:(a + 1) * B],
                                m1b[:, a * B:(a + 1) * B], g2[:, a:a + 1],
                                None, AluOpType.mult)

    # out[a*128 + r, c] = fin[r, a*512 + c]
    nc.sync.dma_start(
        t["out"].ap().rearrange("(a r) c -> r a c", r=128),
        fin[:].rearrange("p (a c) -> p a c", c=B))


def build_program(w_mode=W_MODE, num_devices=NCORES):
    nc = bacc.Bacc("TRN2", target_bir_lowering=False, debug=False,
                   num_devices=num_devices)
    t = {}
    t["x"] = nc.dram_tensor("x", [D, B], BF16, kind="ExternalInput")
    for lyr in range(4):
        t[f"w{lyr}"] = nc.dram_tensor(f"w{lyr}", [128, NK * NS * 128], BF16,
                                      kind="ExternalInput")
    t["brow"] = nc.dram_tensor("brow", [1, 4 * 512], F32R,
                               kind="ExternalInput")
    t["p1m"] = nc.dram_tensor("p1m", [128, L], F32, kind="ExternalInput")
    t["p2r"] = nc.dram_tensor("p2r", [1, B], F32, kind="ExternalInput")
    t["ones"] = nc.dram_tensor("ones", [128, HB], F32R, kind="ExternalInput")
    t["cst"] = nc.dram_tensor("cst", [1, 2], F32R, kind="ExternalInput")
    t["out"] = nc.dram_tensor("out", [B, B], BF16, kind="ExternalOutput")

    with ExitStack() as ctx:
        tc = ctx.enter_context(tile.TileContext(nc))
        _emit(nc, tc, ctx, t)
    nc.compile()
    return nc


def make_in_maps(x1, x2, x1_probs, x2_probs, mlp1_ws, mlp1_bs, mlp2_ws,
                 mlp2_bs):
    import ml_dtypes
    bf = ml_dtypes.bfloat16
    xT = [np.ascontiguousarray(np.asarray(x1, np.float32).T).astype(bf),
          np.ascontiguousarray(np.asarray(x2, np.float32).T).astype(bf)]

    def _tile_w(w, h):
        # [2048, 512] column slice -> [128, (k n) 128] stationary tiles
        w = np.asarray(w, np.float32)[:, 512 * h:512 * (h + 1)]
        w = w.reshape(NK, 128, NS, 128).transpose(1, 0, 2, 3)
        return np.ascontiguousarray(w.reshape(128, NK * NS * 128)).astype(bf)

    ws = [mlp1_ws, mlp2_ws]
    bs = [mlp1_bs, mlp2_bs]
    p1 = np.asarray(x1_probs, np.float32)
    p2 = np.asarray(x2_probs, np.float32)
    in_maps = []
    for c in range(NCORES):
        m = c // 4          # which MLP / side
        h = c % 4           # column shard == label
        brow = np.concatenate(
            [np.asarray(b, np.float32)[512 * h:512 * (h + 1)] for b in bs[m]])
        d = {"x": xT[m],
             "brow": np.ascontiguousarray(brow.reshape(1, 4 * 512)),
             "p1m": np.ascontiguousarray(p1[:, h].reshape(4, 128).T),
             "p2r": np.ascontiguousarray(p2[:, h].reshape(1, B)),
             "ones": np.ones((128, HB), np.float32),
             "cst": np.array([[-512.0, 1.0]], np.float32)}
        for lyr in range(4):
            d[f"w{lyr}"] = _tile_w(ws[m][lyr], h)
        in_maps.append(d)
    return in_maps


_PROGRAM_CACHE = {}


def kernel(x1, x2, x1_probs, x2_probs, mlp1_ws, mlp1_bs, mlp2_ws, mlp2_bs,
           **run_kwargs):
    if "prog" not in _PROGRAM_CACHE:
        _PROGRAM_CACHE["prog"] = build_program()
    nc = _PROGRAM_CACHE["prog"]
    in_maps = make_in_maps(x1, x2, x1_probs, x2_probs, mlp1_ws, mlp1_bs,
                           mlp2_ws, mlp2_bs)
    res = run_bass_kernel_spmd(nc, in_maps, core_ids=list(range(NCORES)),
                               **run_kwargs)
    out = np.stack([np.asarray(res.results[h]["out"], np.float32)
                    for h in range(L)], axis=2)
    kernel.last_results = res
    return np.ascontiguousarray(out)


# revision 9
# speedup vs baseline: 1.7727x; 1.2350x over previous
"""CEAlignment Trainium2 kernel (8 NeuronCores, SPMD).

Sharding (v2, N-sharded MLPs with weight-stationary dataflow):
  - Phase 1 (MLPs): each MLP's weights are column-sharded across its 4 cores
    (core c: MLP c//4, output columns [512*(c%4), +512) of every layer), so
    no core duplicates weight traffic (8.4 MB bf16/core vs 67 MB f32 for the
    data-parallel layout). Weights are the stationary matmul operand
    ([k,n] tiles); activations stay in [feature-part, batch-free] layout the
    whole way through, so there are NO inter-layer transposes. Biases are
    folded in as K=1 matmuls (ones-row moving operand). Activations move
    between layers via a half-batch-pipelined AllGather (groups {0-3},{4-7})
    in bf16, overlapped with compute on the other half.
  - Phase 2: the 512-wide column shard of layer 3 is exactly one label's
    embedding block, so core c already holds label (c%4)'s full q for its
    side. head_normalize is folded into the alignment gram:
    (q1-m1)·(q2-m2) = G - S1*S2/E, scaled by r1*r2 post-matmul. Cores c and
    c+4 exchange raw q (bf16) + (neg-mean, rstd) stat rows via pair
    AllGathers (groups {c, c+4}).
  - Phase 3: align = exp(fixup(G)/sqrt(E)) and a branchless 2-iteration
    Sinkhorn that reproduces the reference's early-exit semantics with the
    convergence selects folded into the per-row/per-col normalization
    factors (g = done ? 1 : factor), so each iteration is only two
    full-width bf16 DVE passes. Cross-partition sums and broadcasts use
    ones-matmuls on the PE. Pair cores compute the same label redundantly;
    cores 0-3's outputs are gathered on the host.

The num_devices==1 build (used by the cost-model timeline) replaces each
collective with local DMAs of equivalent size, as in the v1 kernel.
"""

import math
from contextlib import ExitStack

import numpy as np

import concourse.bacc as bacc
import concourse.bass as bass
import concourse.tile as tile
from concourse import mybir
from concourse.alu_op_type import AluOpType
from concourse.bass_utils import run_bass_kernel_spmd

F32 = mybir.dt.float32
F32R = mybir.dt.float32r
BF16 = mybir.dt.bfloat16
AF = mybir.ActivationFunctionType

B = 512          # batch (both sides)
D = 2048         # input dim
HD = 2048        # hidden dim
E = 512          # embed dim per label
L = 4            # num labels
NCORES = 8
NK = 16          # contraction chunks of 128
NS = 4           # n-tiles of 128 in this core's 512-wide column shard
HB = 256         # half-batch pipeline granule
EPS = 1e-8
ATOL = 0.01
ISQ = 1.0 / math.sqrt(E)
SINKHORN_ITERS = 2
W_MODE = "bf16"  # kept for the test harness printout

LAYER_GROUPS = [[0, 1, 2, 3], [4, 5, 6, 7]]
PAIR_GROUPS = [[0, 4], [1, 5], [2, 6], [3, 7]]


def _allgather(nc, in_ap, out_ap, groups, nrep):
    """AllGather, or equivalent-size local DMAs on the 1-device build."""
    if nc.num_devices == 1:
        n = out_ap.shape[0] // nrep
        for r in range(nrep):
            nc.sync.dma_start(out_ap[r * n:(r + 1) * n], in_ap)
    else:
        nc.gpsimd.collective_compute(
            "AllGather", AluOpType.bypass, replica_groups=groups,
            ins=[in_ap.opt()], outs=[out_ap.opt()])


def _emit(nc, tc, ctx, t):
    const_p = ctx.enter_context(tc.tile_pool(name="const", bufs=1))
    dram_p = ctx.enter_context(
        tc.tile_pool(name="dram", bufs=1, space=bass.MemorySpace.DRAM))

    ones_sb = const_p.tile([128, HB], F32R)
    nc.sync.dma_start(ones_sb[:], t["ones"].ap())
    b_sb = const_p.tile([128, 16], F32)           # bias[l*4+n] per partition
    nc.sync.dma_start(b_sb[:], t["bcol"].ap())
    cst = const_p.tile([1, 2], F32R)              # [-512, 1]
    nc.sync.dma_start(cst[:], t["cst"].ap())
    p1m = const_p.tile([128, L], F32)             # p1 col, chunk-major
    nc.sync.dma_start(p1m[:], t["p1m"].ap())
    p2r = const_p.tile([1, B], F32)               # p2 col as a row
    nc.sync.dma_start(p2r[:], t["p2r"].ap())
    epsb = const_p.tile([1, 1], F32)
    nc.vector.memset(epsb[:], EPS)
    # preload the sqrt act-table set (covers Relu/Copy/Sqrt) while DMAs run;
    # only the exp-set load remains on the phase-3 critical path.
    scr11 = const_p.tile([1, 1], F32)
    nc.scalar.activation(scr11[:], epsb[:], AF.Sqrt)

    ones_col = ones_sb[:, 0:1]                    # [128,1] lhsT: partition sum
    ones_k1 = ones_sb[0:1, 0:128]                 # [1,128] lhsT: bcast to parts

    # DRAM exchange buffers
    ag_in = [[dram_p.tile([E, HB], BF16, tag=f"agi{l}_{h}", name=f"agi{l}_{h}")
              for h in range(2)] for l in range(3)]
    ag_out = [[dram_p.tile([HD, HB], BF16, tag=f"ago{l}_{h}", name=f"ago{l}_{h}")
               for h in range(2)] for l in range(3)]
    pq_in = [dram_p.tile([E, HB], BF16, tag=f"pqi{h}", name=f"pqi{h}")
             for h in range(2)]
    pq_out = [dram_p.tile([2 * E, HB], BF16, tag=f"pqo{h}", name=f"pqo{h}")
              for h in range(2)]
    st_in = dram_p.tile([1, 1024], F32R, tag="sti")
    st_out = dram_p.tile([2, 1024], F32R, tag="sto")

    q1_sb = const_p.tile([128, 4 * B], BF16, tag="q1")   # [e-chunk, batch]
    q2_sb = const_p.tile([128, 4 * B], BF16, tag="q2")
    qh = [const_p.tile([128, NS * HB], BF16, tag=f"qh{h}", name=f"qh{h}")
          for h in range(2)]

    # ---------------- phase 1: MLPs ----------------
    with ExitStack() as p1:
        w_p = p1.enter_context(tc.tile_pool(name="w", bufs=2))
        act_p = p1.enter_context(tc.tile_pool(name="act", bufs=2))
        ps_mm = p1.enter_context(
            tc.tile_pool(name="ps_mm", bufs=6, space=bass.MemorySpace.PSUM))
        ps_q = p1.enter_context(
            tc.tile_pool(name="ps_q", bufs=1, space=bass.MemorySpace.PSUM))
        s_ps = ps_q.tile([1, B], F32, tag="s")
        q_ps = ps_q.tile([1, B], F32, tag="q")

        # x input and layer-0 weights, interleaved so H0 can start early
        x_h = []
        w_half = {}

        def load_w_half(lyr, kh):
            w = w_p.tile([128, 8 * NS * 128], BF16, tag=f"w{kh}",
                         name=f"w{lyr}_{kh}")
            nc.sync.dma_start(
                w[:], t[f"w{lyr}"].ap()[:, kh * 8 * NS * 128:
                                        (kh + 1) * 8 * NS * 128])
            w_half[(lyr, kh)] = w

        def load_x_half(h):
            xt = act_p.tile([128, NK * HB], BF16, tag=f"x{h}", name=f"x{h}")
            nc.sync.dma_start(
                xt[:].rearrange("p (k b) -> p k b", b=HB),
                t["x"].ap().rearrange("(k p) b -> p k b", p=128)
                [:, :, h * HB:(h + 1) * HB])
            x_h.append(xt)

        load_x_half(0)
        load_w_half(0, 0)
        load_w_half(0, 1)
        load_x_half(1)
        rhs = x_h  # per-half rhs, [128, NK*HB], chunk k at [k*HB,(k+1)*HB)

        for lyr in range(4):
            rhs_nxt = []
            for h in range(2):
                if lyr < 3:
                    oo = act_p.tile([128, NS * HB], BF16, tag=f"oo{h}",
                                    name=f"oo{lyr}_{h}")
                else:
                    oo = qh[h]
                pss = [ps_mm.tile([128, HB], F32, tag="mm", name=f"ps{n}")
                       for n in range(NS)]
                for kh in range(2):
                    for n in range(NS):
                        for kk in range(8):
                            k = kh * 8 + kk
                            nc.tensor.matmul(
                                pss[n][:],
                                w_half[(lyr, kh)]
                                [:, (kk * NS + n) * 128:(kk * NS + n + 1) * 128],
                                rhs[h][:, k * HB:(k + 1) * HB],
                                start=(k == 0), stop=(k == NK - 1))
                for n in range(NS):
                    if lyr < 3:
                        nc.scalar.activation(
                            oo[:, n * HB:(n + 1) * HB], pss[n][:], AF.Relu,
                            bias=b_sb[:, lyr * 4 + n:lyr * 4 + n + 1])
                        nc.sync.dma_start(
                            ag_in[lyr][h][n * 128:(n + 1) * 128, :],
                            oo[:, n * HB:(n + 1) * HB])
                    else:
                        nc.vector.tensor_scalar(
                            oo[:, n * HB:(n + 1) * HB], pss[n][:],
                            b_sb[:, 12 + n:13 + n], None, AluOpType.add)
                if lyr < 3:
                    _allgather(nc, ag_in[lyr][h][:], ag_out[lyr][h][:],
                               LAYER_GROUPS, 4)
                    fa = act_p.tile([128, NK * HB], BF16, tag=f"fa{h}",
                                    name=f"fa{lyr}_{h}")
                    for kh in range(2):
                        nc.sync.dma_start(
                            fa[:, kh * 8 * HB:(kh + 1) * 8 * HB]
                            .rearrange("p (k b) -> p k b", b=HB),
                            ag_out[lyr][h][kh * 1024:(kh + 1) * 1024, :]
                            .rearrange("(k p) b -> p k b", p=128))
                    rhs_nxt.append(fa)
                    if lyr < 3:
                        load_w_half(lyr + 1, h)
                else:
                    # stats for this half, then ship q to the pair partner
                    qsq = act_p.tile([128, NS * HB], BF16, tag=f"qsq{h}",
                                     name=f"qsq{h}")
                    nc.vector.tensor_tensor(qsq[:], oo[:], oo[:],
                                            AluOpType.mult)
                    for e4 in range(NS):
                        nc.tensor.matmul(s_ps[0:1, h * HB:(h + 1) * HB],
                                         ones_col,
                                         oo[:, e4 * HB:(e4 + 1) * HB],
                                         start=(e4 == 0), stop=(e4 == NS - 1))
                    for e4 in range(NS):
                        nc.tensor.matmul(q_ps[0:1, h * HB:(h + 1) * HB],
                                         ones_col,
                                         qsq[:, e4 * HB:(e4 + 1) * HB],
                                         start=(e4 == 0), stop=(e4 == NS - 1))
                    nc.sync.dma_start(
                        pq_in[h][:].rearrange("(n p) b -> p n b", p=128),
                        oo[:].rearrange("p (n b) -> p n b", b=HB))
                    _allgather(nc, pq_in[h][:], pq_out[h][:],
                               PAIR_GROUPS, 2)
            rhs = rhs_nxt

        # negm = -S/512 ; r = 1/sqrt((Q - S^2/512)/511 + eps)
        stat2 = const_p.tile([1, 1024], F32R, tag="stat2")
        negm = stat2[:, 0:512]
        rrow = stat2[:, 512:1024]
        nc.scalar.activation(negm, s_ps[:], AF.Copy, scale=-1.0 / E)
        s2row = const_p.tile([1, B], F32R, tag="s2row")
        nc.vector.tensor_tensor(s2row[:], s_ps[:], s_ps[:], AluOpType.mult)
        varr = const_p.tile([1, B], F32R, tag="varr")
        nc.vector.scalar_tensor_tensor(varr[:], s2row[:], -1.0 / E, q_ps[:],
                                       AluOpType.mult, AluOpType.add)
        sdr = const_p.tile([1, B], F32R, tag="sdr")
        nc.scalar.activation(sdr[:], varr[:], AF.Sqrt, bias=epsb[:],
                             scale=1.0 / (E - 1))
        with nc.allow_low_precision("rstd row feeds f32r matmul operands"):
            nc.vector.reciprocal(rrow, sdr[:])
        nc.sync.dma_start(st_in[:], stat2[:])
        _allgather(nc, st_in[:], st_out[:], PAIR_GROUPS, 2)

        # load gathered q into absolute [side] layout
        for h in range(2):
            for (dst, lo) in ((q1_sb, 0), (q2_sb, E)):
                nc.sync.dma_start(
                    dst[:].rearrange("p (e b) -> p e b", b=B)
                    [:, :, h * HB:(h + 1) * HB],
                    pq_out[h][lo:lo + E]
                    .rearrange("(e p) b -> p e b", p=128))

    # ---------------- phase 3: align + sinkhorn ----------------
    stc = const_p.tile([1, 2048], F32R, tag="stc")  # [negm1, r1, negm2, r2]
    nc.sync.dma_start(
        stc[:].rearrange("p (g c) -> p g c", c=1024),
        st_out[:].rearrange("(g p) c -> p g c", p=1))

    snk_p = ctx.enter_context(tc.tile_pool(name="snk", bufs=1))
    ps_g = ctx.enter_context(
        tc.tile_pool(name="ps_g", bufs=2, space=bass.MemorySpace.PSUM))
    ps_bc = ctx.enter_context(
        tc.tile_pool(name="ps_bc", bufs=3, space=bass.MemorySpace.PSUM))
    ps_sm = ctx.enter_context(
        tc.tile_pool(name="ps_sm", bufs=1, space=bass.MemorySpace.PSUM))
    tmp_p = ctx.enter_context(tc.tile_pool(name="tmp", bufs=2))

    # broadcasts of partner-side stats and column extracts of own-side stats
    negm2b = ps_bc.tile([128, B], F32, tag="bc")
    nc.tensor.matmul(negm2b[:], ones_k1, stc[0:1, 1024:1536],
                     start=True, stop=True)
    r2b = ps_bc.tile([128, B], F32, tag="bc")
    nc.tensor.matmul(r2b[:], ones_k1, stc[0:1, 1536:2048],
                     start=True, stop=True)
    misc = ps_sm.tile([128, 16], F32, tag="misc")
    colx = misc[:, 0:8]                           # S1 (a-chunk), r1 (a-chunk)
    for a in range(4):
        nc.tensor.matmul(colx[:, a:a + 1], stc[0:1, a * 128:(a + 1) * 128],
                         cst[0:1, 0:1], start=True, stop=True)
        nc.tensor.matmul(colx[:, 4 + a:5 + a],
                         stc[0:1, 512 + a * 128:512 + (a + 1) * 128],
                         cst[0:1, 1:2], start=True, stop=True)

    # align = exp(((G - S1*S2/E) * r1 * r2) / sqrt(E)), chunk a = batch1 tile
    cur = snk_p.tile([128, 4 * B], BF16, tag="cur")
    for a in range(4):
        g_ps = ps_g.tile([128, B], F32, tag="g")
        for e4 in range(4):
            nc.tensor.matmul(
                g_ps[:], q1_sb[:, e4 * B + a * 128:e4 * B + (a + 1) * 128],
                q2_sb[:, e4 * B:(e4 + 1) * B],
                start=(e4 == 0), stop=(e4 == 3))
        u = tmp_p.tile([128, B], F32R, tag="u")
        nc.vector.scalar_tensor_tensor(u[:], negm2b[:], colx[:, a:a + 1],
                                       g_ps[:], AluOpType.mult, AluOpType.add)
        v = tmp_p.tile([128, B], F32R, tag="v")
        nc.vector.scalar_tensor_tensor(v[:], u[:], colx[:, 4 + a:5 + a],
                                       r2b[:], AluOpType.mult, AluOpType.mult)
        nc.scalar.activation(cur[:, a * B:(a + 1) * B], v[:], AF.Exp,
                             scale=ISQ)

    # ---- sinkhorn: 2 iterations, reference-faithful early-exit blending ----
    def colsum(mat):
        ps = ps_sm.tile([1, B], F32, tag="cs")
        for a in range(4):
            nc.tensor.matmul(ps[:], ones_col, mat[:, a * B:(a + 1) * B],
                             start=(a == 0), stop=(a == 3))
        return ps

    def bcast_row(row_sb):
        ps = ps_bc.tile([128, B], F32, tag="bc")
        nc.tensor.matmul(ps[:], ones_k1, row_sb, start=True, stop=True)
        return ps

    _pcol = [12]

    def bcast_scalar(s11):
        ps = misc[:, _pcol[0]:_pcol[0] + 1]
        _pcol[0] += 1
        nc.tensor.matmul(ps, ones_k1, s11, start=True, stop=True)
        return ps

    def row_norm_factors(rs4, tag):
        """f4 = p1 / (rowsum + eps); grow = all-rows-converged flag [1,1]."""
        rr4 = snk_p.tile([128, L], F32, tag=f"rr{tag}")
        nc.vector.reciprocal(rr4[:], rs4[:])
        f4 = snk_p.tile([128, L], F32, tag=f"f4{tag}")
        nc.vector.tensor_tensor(f4[:], rr4[:], p1m[:], AluOpType.mult)
        dev = snk_p.tile([128, L], F32, tag=f"dv{tag}")
        nc.vector.tensor_tensor(dev[:], rs4[:], p1m[:], AluOpType.subtract)
        dsq = snk_p.tile([128, L], F32, tag=f"dq{tag}")
        nc.vector.tensor_tensor(dsq[:], dev[:], dev[:], AluOpType.mult)
        dcl = snk_p.tile([128, L], F32R, tag=f"dc{tag}")
        nc.vector.tensor_scalar(dcl[:], dsq[:], ATOL * ATOL, 0.0,
                                AluOpType.subtract, AluOpType.max)
        pv = misc[0:1, 8:12]
        nc.tensor.matmul(pv, ones_col, dcl[:], start=True, stop=True)
        vr = snk_p.tile([1, 1], F32, tag=f"vr{tag}")
        s14 = snk_p.tile([1, L], F32, tag=f"s14{tag}")
        nc.vector.tensor_scalar(s14[:], pv, 0.0, None, AluOpType.add,
                                AluOpType.add, accum_out=vr[:])
        grow = snk_p.tile([1, 1], F32R, tag=f"gr{tag}")
        nc.vector.tensor_scalar(grow[:], vr[:], 1e-30, None, AluOpType.is_le)
        return f4, grow

    # iteration 1: column normalize
    cs1 = colsum(cur)
    csr = snk_p.tile([1, B], F32, tag="csr")
    nc.vector.reciprocal(csr[:], cs1[:])
    srow = snk_p.tile([1, B], F32R, tag="srow")
    nc.vector.tensor_tensor(srow[:], csr[:], p2r[:], AluOpType.mult)
    sb_ps = bcast_row(srow[:])
    sful = snk_p.tile([128, B], BF16, tag="sful")
    nc.scalar.copy(sful[:], sb_ps[:])
    m1 = snk_p.tile([128, 4 * B], BF16, tag="m1")
    rs4 = snk_p.tile([128, L], F32, tag="rs4")
    for a in range(4):
        nc.vector.scalar_tensor_tensor(
            m1[:, a * B:(a + 1) * B], cur[:, a * B:(a + 1) * B], 1.0, sful[:],
            AluOpType.mult, AluOpType.mult, accum_out=rs4[:, a:a + 1])
    # row normalize, folded with the row_ok select: g1 = row_ok ? 1 : f4
    f4, grow = row_norm_factors(rs4, "1")
    pg = bcast_scalar(grow[:])
    d4 = snk_p.tile([128, L], F32, tag="d4")
    nc.vector.tensor_scalar(d4[:], f4[:], -1.0, 1.0, AluOpType.mult,
                            AluOpType.add)
    g1 = snk_p.tile([128, L], F32, tag="g1")
    nc.vector.scalar_tensor_tensor(g1[:], d4[:], pg, f4[:],
                                   AluOpType.mult, AluOpType.add)
    cur2 = snk_p.tile([128, 4 * B], BF16, tag="cur2")
    for a in range(4):
        nc.vector.tensor_scalar(cur2[:, a * B:(a + 1) * B],
                                m1[:, a * B:(a + 1) * B], g1[:, a:a + 1],
                                None, AluOpType.mult)

    # iteration 2 (no-op via factor blending if iteration 1 converged)
    cs2 = colsum(cur2)
    cd = snk_p.tile([1, B], F32, tag="cd")
    nc.vector.tensor_tensor(cd[:], cs2[:], p2r[:], AluOpType.subtract)
    cdq = snk_p.tile([1, B], F32, tag="cdq")
    nc.vector.tensor_tensor(cdq[:], cd[:], cd[:], AluOpType.mult)
    vc = snk_p.tile([1, 1], F32, tag="vc")
    cdc = snk_p.tile([1, B], F32, tag="cdc")
    nc.vector.tensor_scalar(cdc[:], cdq[:], ATOL * ATOL, 0.0,
                            AluOpType.subtract, AluOpType.max,
                            accum_out=vc[:])
    gcol = snk_p.tile([1, 1], F32, tag="gcol")
    nc.vector.tensor_scalar(gcol[:], vc[:], 1e-30, None, AluOpType.is_le)
    done1 = snk_p.tile([1, 1], F32R, tag="done1")
    nc.vector.tensor_tensor(done1[:], grow[:], gcol[:], AluOpType.max)
    pd = bcast_scalar(done1[:])
    csr2 = snk_p.tile([1, B], F32, tag="csr2")
    nc.vector.reciprocal(csr2[:], cs2[:])
    srow2 = snk_p.tile([1, B], F32R, tag="srow2")
    nc.vector.tensor_tensor(srow2[:], csr2[:], p2r[:], AluOpType.mult)
    sb2_ps = bcast_row(srow2[:])
    ssb = snk_p.tile([128, B], BF16, tag="ssb")
    nc.scalar.copy(ssb[:], sb2_ps[:])
    # s' = done1 ? 1 : srow2  (full-width blend of the column factor)
    t1 = snk_p.tile([128, B], BF16, tag="t1")
    nc.vector.tensor_scalar(t1[:], ssb[:], -1.0, 1.0, AluOpType.mult,
                            AluOpType.add)
    sp = snk_p.tile([128, B], BF16, tag="sp")
    nc.vector.scalar_tensor_tensor(sp[:], t1[:], pd, ssb[:],
                                   AluOpType.mult, AluOpType.add)
    m1b = snk_p.tile([128, 4 * B], BF16, tag="m1b")
    rs4b = snk_p.tile([128, L], F32, tag="rs4b")
    for a in range(4):
        nc.vector.scalar_tensor_tensor(
            m1b[:, a * B:(a + 1) * B], cur2[:, a * B:(a + 1) * B], 1.0, sp[:],
            AluOpType.mult, AluOpType.mult, accum_out=rs4b[:, a:a + 1])
    f4b, grow2 = row_norm_factors(rs4b, "2")
    og = snk_p.tile([1, 1], F32R, tag="og")
    nc.vector.tensor_tensor(og[:], done1[:], grow2[:], AluOpType.max)
    pg2 = bcast_scalar(og[:])
    d4b = snk_p.tile([128, L], F32, tag="d4b")
    nc.vector.tensor_scalar(d4b[:], f4b[:], -1.0, 1.0, AluOpType.mult,
                            AluOpType.add)
    g2 = snk_p.tile([128, L], F32, tag="g2")
    nc.vector.scalar_tensor_tensor(g2[:], d4b[:], pg2, f4b[:],
                                   AluOpType.mult, AluOpType.add)
    fin = snk_p.tile([128, 4 * B], BF16, tag="fin")
    for a in range(4):
        nc.vector.tensor_scalar(fin[:, a * B:(a + 1) * B],
                                m1b[:, a * B:(a + 1) * B], g2[:, a:a + 1],
                                None, AluOpType.mult)

    # out[a*128 + r, c] = fin[r, a*512 + c]
    nc.sync.dma_start(
        t["out"].ap().rearrange("(a r) c -> r a c", r=128),
        fin[:].rearrange("p (a c) -> p a c", c=B))


def build_program(w_mode=W_MODE, num_devices=NCORES):
    nc = bacc.Bacc("TRN2", target_bir_lowering=False, debug=False,
                   num_devices=num_devices)
    t = {}
    t["x"] = nc.dram_tensor("x", [D, B], BF16, kind="ExternalInput")
    for lyr in range(4):
        t[f"w{lyr}"] = nc.dram_tensor(f"w{lyr}", [128, NK * NS * 128], BF16,
                                      kind="ExternalInput")
    t["bcol"] = nc.dram_tensor("bcol", [128, 16], F32, kind="ExternalInput")
    t["p1m"] = nc.dram_tensor("p1m", [128, L], F32, kind="ExternalInput")
    t["p2r"] = nc.dram_tensor("p2r", [1, B], F32, kind="ExternalInput")
    t["ones"] = nc.dram_tensor("ones", [128, HB], F32R, kind="ExternalInput")
    t["cst"] = nc.dram_tensor("cst", [1, 2], F32R, kind="ExternalInput")
    t["out"] = nc.dram_tensor("out", [B, B], BF16, kind="ExternalOutput")

    with ExitStack() as ctx:
        tc = ctx.enter_context(tile.TileContext(nc))
        _emit(nc, tc, ctx, t)
    nc.compile()
    return nc


def make_in_maps(x1, x2, x1_probs, x2_probs, mlp1_ws, mlp1_bs, mlp2_ws,
                 mlp2_bs):
    import ml_dtypes
    bf = ml_dtypes.bfloat16
    xT = [np.ascontiguousarray(np.asarray(x1, np.float32).T).astype(bf),
          np.ascontiguousarray(np.asarray(x2, np.float32).T).astype(bf)]

    def _tile_w(w, h):
        # [2048, 512] column slice -> [128, (k n) 128] stationary tiles
        w = np.asarray(w, np.float32)[:, 512 * h:512 * (h + 1)]
        w = w.reshape(NK, 128, NS, 128).transpose(1, 0, 2, 3)
        return np.ascontiguousarray(w.reshape(128, NK * NS * 128)).astype(bf)

    ws = [mlp1_ws, mlp2_ws]
    bs = [mlp1_bs, mlp2_bs]
    p1 = np.asarray(x1_probs, np.float32)
    p2 = np.asarray(x2_probs, np.float32)
    in_maps = []
    for c in range(NCORES):
        m = c // 4          # which MLP / side
        h = c % 4           # column shard == label
        bcol = np.stack(
            [np.asarray(b, np.float32)[512 * h + n * 128:512 * h + (n + 1) * 128]
             for b in bs[m] for n in range(NS)], axis=1)
        d = {"x": xT[m],
             "bcol": np.ascontiguousarray(bcol),
             "p1m": np.ascontiguousarray(p1[:, h].reshape(4, 128).T),
             "p2r": np.ascontiguousarray(p2[:, h].reshape(1, B)),
             "ones": np.ones((128, HB), np.float32),
             "cst": np.array([[-512.0, 1.0]], np.float32)}
        for lyr in range(4):
            d[f"w{lyr}"] = _tile_w(ws[m][lyr], h)
        in_maps.append(d)
    return in_maps


_PROGRAM_CACHE = {}


def kernel(x1, x2, x1_probs, x2_probs, mlp1_ws, mlp1_bs, mlp2_ws, mlp2_bs,
           **run_kwargs):
    if "prog" not in _PROGRAM_CACHE:
        _PROGRAM_CACHE["prog"] = build_program()
    nc = _PROGRAM_CACHE["prog"]
    in_maps = make_in_maps(x1, x2, x1_probs, x2_probs, mlp1_ws, mlp1_bs,
                           mlp2_ws, mlp2_bs)
    res = run_bass_kernel_spmd(nc, in_maps, core_ids=list(range(NCORES)),
                               **run_kwargs)
    out = np.stack([np.asarray(res.results[h]["out"], np.float32)
                    for h in range(L)], axis=2)
    kernel.last_results = res
    return np.ascontiguousarray(out)
